# revision 1
# baseline (speedup 1.0000x reference)
"""Trainium2 Bass kernel for multi-head attention (B=2, S=2048, D=1024, H=16).

Sharding: data-parallel over query rows. Core c handles batch b=c//4 and
query rows [512*(c%4), 512*(c%4+1)). Each core computes K/V projections for
all heads over the full sequence (duplicated across the 4 cores sharing a
batch), Q projection for its 512 rows, attention, and the output projection
for its rows. No cross-core communication.

Layouts (all chosen so the contraction dim lands on SBUF partitions and no
on-device transposes are needed):
  xT   [8,128,2048]  x[b] transposed (d on partitions), s-axis rolled so this
                     core's q-block sits at columns 0:512
  kT   per 2-head group [128, 2048]: partitions = (head parity)*64 + dh
  v    per s-chunk [128, 4, 65]: v for 4 heads + denominator column
  scores^T [s, q] so the attn@v contraction needs no transpose; softmax
  denominator comes from the extra column of v (M=65 matmul output row 64).

Padding mask: V rows (and the denominator column) are multiplied by the 0/1
key mask, so masked keys contribute exactly 0 to both the numerator and the
softmax denominator — identical to the reference's -1e9 score masking, and
it keeps the exp activation bias-free so two score chunks share one
[128, 1024] exp op. Softmax skips max-subtraction (scores are ~N(0,1) after
the 1/8 scale; exp cannot overflow fp32).

All matmuls use float32r (TF32-like, full PE rate at N>=256; inputs are
pre-rounded on the host) with K=128 (scores use K=128 with the unused
head-half of q zeroed so the PE never switches tiling modes).
"""

import os
import sys

sys.path.insert(0, "/opt/trn_rl_repo")

import numpy as np

B, S, D, H, DH = 2, 2048, 1024, 16, 64
NCORES = 8
CPB = NCORES // B       # cores per batch
QB = S // CPB           # 512 query rows per core
P = 128
DCH = D // P            # 8 contraction chunks
SC = S // P             # 16 s-chunks
NEG = -1e9

_compiled = {}
LAST_RESULTS = None
ABLATE = set()   # debug: {"kv1","exp_copy","b1","c1"} cripple phases for HW bisection
UNROLL = 1       # debug: repeat the whole body N times inside one NEFF


def _build_program():
    import concourse.bass as bass
    import concourse.mybir as mybir
    import concourse.tile as tile
    from concourse import bacc

    f32 = mybir.dt.float32
    f32r = mybir.dt.float32r
    AF = mybir.ActivationFunctionType
    OP = mybir.AluOpType


    nc = bacc.Bacc(
        "TRN2", target_bir_lowering=False, debug=False,
        num_devices=NCORES,
    )

    xT = nc.dram_tensor("xT", [DCH, P, S], f32r, kind="ExternalInput")
    wq = nc.dram_tensor("wq", [H // 2, P, DCH, P], f32r, kind="ExternalInput")
    wk = nc.dram_tensor("wk", [H // 2, P, DCH, P], f32r, kind="ExternalInput")
    wv = nc.dram_tensor("wv", [H // 4, P, DCH, 256], f32r, kind="ExternalInput")
    woT = nc.dram_tensor("woT", [DCH, P, D], f32r, kind="ExternalInput")
    bq = nc.dram_tensor("bq", [P, H // 2], f32, kind="ExternalInput")
    bk = nc.dram_tensor("bk", [P, H // 2], f32, kind="ExternalInput")
    bv = nc.dram_tensor("bv", [1, D], f32, kind="ExternalInput")
    bo = nc.dram_tensor("bo", [1, D], f32, kind="ExternalInput")
    maskT = nc.dram_tensor("maskT", [P, SC], f32, kind="ExternalInput")
    out = nc.dram_tensor("out", [QB, D], f32, kind="ExternalOutput")

    with tile.TileContext(nc) as tc:
        with (
            tc.tile_pool(name="const", bufs=1) as constp,
            tc.tile_pool(name="big", bufs=DCH) as bigp,
            tc.tile_pool(name="w", bufs=2) as wpool,
            tc.tile_pool(name="kt", bufs=2) as ktpool,
            tc.tile_pool(name="va", bufs=SC) as vpool,
            tc.tile_pool(name="qtz", bufs=4) as qpool,
            tc.tile_pool(name="pt", bufs=4) as ptpool,
            tc.tile_pool(name="cat", bufs=1) as catp,
            tc.tile_pool(name="rr", bufs=2) as rpool,
            tc.tile_pool(name="osb", bufs=2) as outp,
            tc.tile_pool(name="pp", bufs=2, space="PSUM") as pp,
            tc.tile_pool(name="psc", bufs=2, space="PSUM") as psc,
            tc.tile_pool(name="po", bufs=2, space="PSUM") as pop,
        ):
            # ---- constants
            bq_sb = constp.tile([P, H // 2], f32, tag="bq")
            nc.sync.dma_start(out=bq_sb[:], in_=bq[:])
            bk_sb = constp.tile([P, H // 2], f32, tag="bk")
            nc.sync.dma_start(out=bk_sb[:], in_=bk[:])
            mask_sb = constp.tile([P, SC], f32, tag="mask")
            nc.sync.dma_start(out=mask_sb[:], in_=maskT[:])
            bv_src = constp.tile([1, D], f32, tag="bvs")
            nc.sync.dma_start(out=bv_src[:], in_=bv[:])
            bo_src = constp.tile([1, D], f32, tag="bos")
            nc.sync.dma_start(out=bo_src[:], in_=bo[:])
            bv_rep = constp.tile([P, D], f32, tag="bvr")
            nc.gpsimd.partition_broadcast(bv_rep[:], bv_src[:])
            bo_rep = constp.tile([P, D], f32, tag="bor")
            nc.gpsimd.partition_broadcast(bo_rep[:], bo_src[:])

            for rep in range(UNROLL):
              concat = catp.tile([P, DCH, QB], f32r, tag="cat",
                                 name=f"cat{rep}")

              # ---- x^T resident in SBUF (8 chunks of [128, 2048])
              xt = []
              for d in range(DCH):
                  t = bigp.tile([P, S], f32r, tag="big", name=f"xt{rep}_{d}")
                  nc.sync.dma_start(out=t[:], in_=xT[d])
                  xt.append(t)

              NW = 4          # waves
              HPW = H // NW   # heads per wave

              for wave in range(NW):
                  groups = [2 * wave, 2 * wave + 1]
                  # ---- A: kT projection (2-head groups, output [2*64 dh, s])
                  kt = []
                  for gl, g in enumerate(groups):
                      wk_t = wpool.tile([P, DCH, P], f32r, tag="wk")
                      nc.sync.dma_start(out=wk_t[:], in_=wk[g])
                      ktile = ktpool.tile([P, S], f32r, tag="kt")
                      DR = 1 if "kv1" in ABLATE else DCH
                      for sb in range(4):
                          ps = pp.tile([P, 512], f32, tag="pp")
                          for d in range(DR):
                              nc.tensor.matmul(
                                  ps[:],
                                  wk_t[:, d, :],
                                  xt[d][:, sb * 512:(sb + 1) * 512],
                                  start=(d == 0),
                                  stop=(d == DR - 1),
                              )
                          nc.vector.tensor_scalar_add(
                              ktile[:, sb * 512:(sb + 1) * 512], ps[:],
                              bk_sb[:, g:g + 1],
                          )
                      kt.append(ktile)

                  # ---- A: v projection (4 heads at once, natural [s, 4*64])
                  wv_t = wpool.tile([P, DCH, 256], f32r, tag="wv")
                  nc.sync.dma_start(out=wv_t[:], in_=wv[wave])
                  va = []
                  for sc in range(SC):
                      vt = vpool.tile([P, HPW, 65], f32r, tag="va")
                      ps = pp.tile([P, 512], f32, tag="pp",
                                   name=f"vps_{wave}_{sc}")[:, 0:256]
                      DR = 1 if "kv1" in ABLATE else DCH
                      for d in range(DR):
                          nc.tensor.matmul(
                              ps[:],
                              xt[d][:, sc * P:(sc + 1) * P],
                              wv_t[:, d, :],
                              start=(d == 0),
                              stop=(d == DR - 1),
                          )
                      ps_r = ps.rearrange("p (h e) -> p h e", e=64)
                      nc.vector.tensor_tensor(
                          vt[:, :, 0:64],
                          ps_r,
                          bv_rep[:, wave * 256:(wave + 1) * 256].rearrange(
                              "p (h e) -> p h e", e=64),
                          OP.add,
                      )
                      # zero out masked key rows: masked s contributes 0 to
                      # both numerator and denominator (same as -1e9 scores)
                      nc.vector.tensor_scalar(
                          vt[:, :, 0:64], vt[:, :, 0:64],
                          mask_sb[:, sc:sc + 1], None, OP.mult,
                      )
                      # denominator column = mask (1 for valid, 0 for padded)
                      nc.vector.tensor_scalar(
                          vt[:, :, 64:65], ps_r[:, :, 0:1], 0.0,
                          mask_sb[:, sc:sc + 1], OP.mult, OP.add,
                      )
                      va.append(vt)

                  # ---- A: q projection for this wave's groups; per head a
                  # [128, 512] tile with the other head-half zeroed (keeps the
                  # scores matmul at K=128, no PE tiling-mode switches).
                  qtz = []
                  for gl, g in enumerate(groups):
                      wq_t = wpool.tile([P, DCH, P], f32r, tag="wq")
                      nc.sync.dma_start(out=wq_t[:], in_=wq[g])
                      ps = pp.tile([P, 512], f32, tag="pp")
                      for d in range(DCH):
                          nc.tensor.matmul(
                              ps[:],
                              wq_t[:, d, :],
                              xt[d][:, 0:QB],
                              start=(d == 0),
                              stop=(d == DCH - 1),
                          )
                      for par in range(2):
                          qz = qpool.tile([P, QB], f32r, tag="qtz")
                          lo, hi = par * 64, (par + 1) * 64
                          olo, ohi = (1 - par) * 64, (2 - par) * 64
                          nc.vector.tensor_scalar(
                              qz[olo:ohi, :], ps[olo:ohi, :], 0.0, None,
                              OP.mult,
                          )
                          nc.vector.tensor_scalar_add(
                              qz[lo:hi, :], ps[lo:hi, :], bq_sb[lo:hi, g:g + 1],
                          )
                          qtz.append(qz)

                  # ---- B: attention per head
                  for hl in range(HPW):
                      gl, par = hl // 2, hl % 2
                      po_t = pop.tile([P, QB], f32, tag="po")
                      pts = {}

                      def emit_scores_pair(pc):
                          # two s-chunks -> one [128, 1024] psum (2 banks),
                          # one exp over both (amortizes ACT op overhead)
                          sps = psc.tile([P, 2, QB], f32, tag="ps")
                          for j in range(2):
                              sc = 2 * pc + j
                              nc.tensor.matmul(
                                  sps[:, j, :],
                                  kt[gl][:, sc * P:(sc + 1) * P],
                                  qtz[2 * gl + par][:],
                                  start=True,
                                  stop=True,
                              )
                          pt = ptpool.tile([P, 2, QB], f32r, tag="pt")
                          if "exp_copy" in ABLATE:
                              nc.vector.tensor_scalar(
                                  pt[:], sps[:], 0.125, None, OP.mult)
                          else:
                              nc.scalar.activation(
                                  pt[:], sps[:], AF.Exp,
                                  bias=0.0, scale=0.125,
                              )
                          pts[pc] = pt

                      def emit_o(pc):
                          pt = pts.pop(pc)
                          for j in range(2):
                              sc = 2 * pc + j
                              nc.tensor.matmul(
                                  po_t[0:65, :],
                                  va[sc][:, hl, :],
                                  pt[:, j, :],
                                  start=(sc == 0),
                                  stop=(sc == SC - 1),
                              )

                      NP = SC // 2
                      if "b1" in ABLATE:
                          emit_scores_pair(0)
                          pt = pts.pop(0)
                          nc.tensor.matmul(
                              po_t[0:65, :], va[0][:, hl, :], pt[:, 0, :],
                              start=True, stop=True)
                      else:
                          emit_scores_pair(0)
                          emit_scores_pair(1)
                          for pc in range(2, NP):
                              emit_o(pc - 2)
                              emit_scores_pair(pc)
                          emit_o(NP - 2)
                          emit_o(NP - 1)

                      # normalize: row 64 of po_t is the softmax denominator
                      den = rpool.tile([65, QB], f32, tag="den")
                      nc.vector.reciprocal(den[64:65, :], po_t[64:65, :])
                      # partition_broadcast requires a base-0 input on HW
                      den0 = rpool.tile([1, QB], f32, tag="den0")
                      nc.sync.dma_start(out=den0[:], in_=den[64:65, :])
                      rep = rpool.tile([P, QB], f32, tag="rep")
                      nc.gpsimd.partition_broadcast(rep[:], den0[0:1, :])
                      cslot = wave * 2 + gl
                      if par == 0:
                          nc.vector.tensor_tensor(
                              concat[0:64, cslot, :], po_t[0:64, :],
                              rep[0:64, :], OP.mult,
                          )
                      else:
                          tmp = rpool.tile([64, QB], f32r, tag="tmp")
                          nc.vector.tensor_tensor(
                              tmp[:], po_t[0:64, :], rep[0:64, :], OP.mult,
                          )
                          nc.sync.dma_start(
                              out=concat[64:P, cslot, :], in_=tmp[:],
                          )

              # ---- C: output projection (contraction over h*dh in 8 chunks)
              wo_sb = []
              for c in range(DCH):
                  t = bigp.tile([P, D], f32r, tag="big")
                  nc.sync.dma_start(out=t[:], in_=woT[c])
                  wo_sb.append(t)
              for qt_i in range(QB // P):
                  for eb in range(2):
                      ps = pp.tile([P, 512], f32, tag="pp")
                      CR = 1 if "c1" in ABLATE else DCH
                      for c in range(CR):
                          nc.tensor.matmul(
                              ps[:],
                              concat[:, c, qt_i * P:(qt_i + 1) * P],
                              wo_sb[c][:, eb * 512:(eb + 1) * 512],
                              start=(c == 0),
                              stop=(c == CR - 1),
                          )
                      osb = outp.tile([P, 512], f32, tag="osb")
                      nc.vector.tensor_tensor(
                          osb[:], ps[:], bo_rep[:, eb * 512:(eb + 1) * 512],
                          OP.add,
                      )
                      nc.sync.dma_start(
                          out=out[qt_i * P:(qt_i + 1) * P,
                                  eb * 512:(eb + 1) * 512],
                          in_=osb[:],
                      )

    nc.compile()
    nc.finalize()
    return nc


def _round_fp32r(a):
    """Round fp32 values to fp32r (TF32-like, 11-bit mantissa, RNE)."""
    u = np.ascontiguousarray(a, dtype=np.float32).view(np.uint32).astype(np.uint64)
    r = ((u + 0x7FF + ((u >> 12) & 1)) & 0xFFFFF000).astype(np.uint32)
    return r.view(np.float32).reshape(a.shape)


def prep_inputs(x, pad_mask, wq, wk, wv, bq, bk, bv, wo, bo):
    """Build per-core input maps (host-side shard + layout prep)."""
    x = np.ascontiguousarray(np.asarray(x, dtype=np.float32))
    pad_mask = np.asarray(pad_mask)
    wq = np.asarray(wq, dtype=np.float32)
    wk = np.asarray(wk, dtype=np.float32)
    wv = np.asarray(wv, dtype=np.float32)
    bq = np.asarray(bq, dtype=np.float32)
    bk = np.asarray(bk, dtype=np.float32)
    bv = np.asarray(bv, dtype=np.float32)
    wo = np.asarray(wo, dtype=np.float32)
    bo = np.asarray(bo, dtype=np.float32)

    # weights: [H, D, DH] -> [d, h*dh] (h-major columns)
    def stack_groups(w, gsz):
        ws = np.ascontiguousarray(w.transpose(1, 0, 2).reshape(D, D))
        # -> [group, di, do, gsz*DH]
        m = gsz * DH
        arr = ws.reshape(DCH, P, H // gsz, m).transpose(2, 1, 0, 3)
        return np.ascontiguousarray(arr)

    wq_dev = _round_fp32r(stack_groups(wq, 2))
    wk_dev = _round_fp32r(stack_groups(wk, 2))
    wv_dev = _round_fp32r(stack_groups(wv, 4))
    woT_dev = _round_fp32r(np.ascontiguousarray(wo.T).reshape(DCH, P, D))
    bq_dev = np.ascontiguousarray(bq.reshape(H // 2, P).T)
    bk_dev = np.ascontiguousarray(bk.reshape(H // 2, P).T)
    bv_dev = np.ascontiguousarray(bv.reshape(1, D))
    bo_dev = np.ascontiguousarray(bo.reshape(1, D))

    in_maps = []
    for c in range(NCORES):
        b, qo = c // CPB, c % CPB
        # transpose + roll the s axis so this core's q rows are cols 0:QB
        xt = x[b].T  # [D, S]
        xt = np.roll(xt, -qo * QB, axis=1)
        xt_dev = _round_fp32r(np.ascontiguousarray(xt)).reshape(DCH, P, S)
        m01 = (pad_mask[b] != 0).astype(np.float32)
        m01 = np.roll(m01, -qo * QB)
        maskT_dev = np.ascontiguousarray(m01.reshape(SC, P).T)
        in_maps.append({
            "xT": xt_dev, "wq": wq_dev, "wk": wk_dev, "wv": wv_dev,
            "woT": woT_dev, "bq": bq_dev, "bk": bk_dev, "bv": bv_dev,
            "bo": bo_dev, "maskT": maskT_dev,
        })
    return in_maps


def kernel(**inputs):
    global LAST_RESULTS
    from concourse.bass_utils import run_bass_kernel_spmd

    if "nc" not in _compiled:
        _compiled["nc"] = _build_program()
    nc = _compiled["nc"]

    in_maps = prep_inputs(**inputs)
    res = run_bass_kernel_spmd(
        nc, in_maps, list(range(NCORES)),
        trace=bool(os.environ.get("BASS_TRACE")),
    )
    LAST_RESULTS = res

    out = np.empty((B, S, D), dtype=np.float32)
    for c in range(NCORES):
        b, qo = c // CPB, c % CPB
        out[b, qo * QB:(qo + 1) * QB, :] = res.results[c]["out"]
    return out

